# revision 22
# baseline (speedup 1.0000x reference)
"""Multi-head attention (B=4, S=2048, D=1024, H=16, HD=64) on 8 TRN2 NeuronCores.

Sharding: core c handles batch b=c//2 and head-group g=c%2 (8 heads).
W_q/W_k/W_v column-sharded, W_o row-sharded; the two partial outputs per
batch are summed on the host.

Per-core kernel (matmuls bf16, accumulation fp32 in PSUM):
  Projections: x^T [D, S] bf16 tiles so TensorE contracts over D directly.
  qT/kT [128, S] per head-pair (two heads stacked on partitions);
  v [S, 8*65] with a ones column per head.

  Attention in transposed layout: scoresT[j, i] = k q^T via two K=64
  matmuls per j-tile (tile_position stacks the pair on the PE array);
  exp on ScalarE (scale=1/8 folded in); causal structure from the mask at
  build time (fully-masked 128x128 blocks skipped, mixed blocks zeroed by
  a 0/1 valid matrix).

  ctx is computed in the flipped orientation: lhsT = attn tile [j, i-chunk],
  rhs = [v_h | 1] [j, 65] streaming only 65 columns per (head, j-tile,
  i-chunk) at K=128 - half the PE cost of streaming the i range at M=65.
  The ones column makes PSUM col 64 the softmax denominator, which is now a
  per-partition scalar: one DVE reciprocal + tensor_scalar multiply
  normalizes while evacuating (no DRAM bounce).  A DMA xbar transpose
  ([i, (h,f)] -> [(h,f), i]) builds the pair-stacked ctxT for the output
  projection without touching PE.

  Emission is i-block-major with a credit-based filler queue: projection
  and output-projection matmul units are interleaved into the exp-gated
  attention stream so the in-order PE queue always has work while ScalarE
  computes exponentials.
"""

import sys

sys.path.insert(0, "/opt/trn_rl_repo")

import numpy as np
import ml_dtypes

import concourse.bacc as bacc
import concourse.tile as tile
from concourse import mybir

BF16 = ml_dtypes.bfloat16
F32 = mybir.dt.float32
BF = mybir.dt.bfloat16

B, S, D, H, HD = 4, 2048, 1024, 16, 64
G = 2              # head groups (cores per batch)
HPG = H // G       # 8 heads per group
NPAIR = HPG // 2   # 4 head pairs
FB = HPG * HD      # 512 projection cols per group
BLK = 128          # j-tile size
IBW = 512          # i-block width
NIB = S // IBW     # 4 i-blocks
NJT = S // BLK     # 16 j-tiles
NDT = D // BLK     # 8 contraction tiles
NST = S // BLK     # 16 s-tiles for the output projection
VW = HD + 1        # 65: v plus ones column
NCH = IBW // BLK   # 4 i-chunks per i-block
EXP_SCALE = 1.0 / np.sqrt(np.float32(HD))

# cost-model constants for the filler credit heuristic (ns)
PE_NS_PER_COL = 1.0 / 2.4
ACT_NS_PER_COL = 1.0 / 1.2
ACT_FIXED = 185.0
MM512 = 512 * PE_NS_PER_COL


def classify_mask(mask: np.ndarray):
    """Block states over the *transposed* mask grid: state[jt][it] for the
    region j in [128jt,..), i in [128it,..).  0=all valid, 1=all masked,
    2=mixed."""
    m = np.asarray(mask)
    blocks = m.reshape(NJT, BLK, NJT, BLK).transpose(0, 2, 1, 3)  # [it, jt, i, j]
    anym = blocks.any(axis=(2, 3))
    allm = blocks.all(axis=(2, 3))
    states = np.where(allm, 1, np.where(anym, 2, 0)).astype(np.int8)
    return states.T  # index [jt, it]


def build_plan(states: np.ndarray):
    """Per i-block: list of (jt, c0, c1, mixed_ks).  c0/c1 bound the valid
    i-columns (relative to the block) at 128 granularity; mixed_ks are the
    128-col sub-blocks needing a 0/1 multiply (includes interior all-masked
    sub-blocks, which get an all-zero valid matrix)."""
    plan = []
    mixed_slots = {}
    for ib in range(NIB):
        its = list(range(4 * ib, 4 * ib + 4))
        jts = []
        for jt in range(NJT):
            sub = [int(states[jt, it]) for it in its]
            nz = [k for k, st in enumerate(sub) if st != 1]
            if not nz:
                continue
            k0, k1 = nz[0], nz[-1]
            mixed = [k for k in range(k0, k1 + 1) if sub[k] != 0]
            for k in mixed:
                mixed_slots.setdefault((jt, its[k]), len(mixed_slots))
            jts.append((jt, k0 * BLK, (k1 + 1) * BLK, mixed))
        assert jts, "fully-masked i-block not supported"
        plan.append(jts)
    return plan, mixed_slots


def plan_key(plan, mixed_slots):
    return (
        tuple(
            tuple((jt, c0, c1, tuple(mk)) for jt, c0, c1, mk in jts) for jts in plan
        ),
        tuple(sorted(mixed_slots.items())),
    )


class Feeder:
    """Ordered queue of PE filler-work generators.  Each generator yields
    (pe_cost_ns, emit_fn) items.  spend() emits items against an ACT-vs-PE
    credit; drain_until() force-emits everything up to a named unit so its
    consumers can be issued afterwards."""

    def __init__(self):
        self.queue = []
        self.credit = 0.0
        self.emitted = set()

    def add(self, name, gen):
        self.queue.append((name, gen))

    def _emit_one(self):
        while self.queue:
            name, gen = self.queue[0]
            try:
                cost, fn = next(gen)
            except StopIteration:
                self.emitted.add(name)
                self.queue.pop(0)
                continue
            fn()
            return cost
        return None

    def drain_until(self, name):
        while name not in self.emitted:
            if self._emit_one() is None:
                raise RuntimeError(f"feeder exhausted before {name}")

    def drain_all(self):
        while self._emit_one() is not None:
            pass

    def spend(self, ns, cap=3000.0):
        self.credit = min(self.credit + ns, cap)
        while self.credit > 0 and self.queue:
            cost = self._emit_one()
            if cost is None:
                return
            self.credit -= cost


def build_nc(plan, mixed_slots):
    nvb = max(1, len(mixed_slots))
    nc = bacc.Bacc("TRN2", target_bir_lowering=False, debug=False, num_devices=8)

    xqT = nc.dram_tensor("xqT", [D, S], BF, kind="ExternalInput").ap()
    xkT = nc.dram_tensor("xkT", [D, S], BF, kind="ExternalInput").ap()
    xvS = nc.dram_tensor("xvS", [NJT, NDT * BLK, BLK], BF, kind="ExternalInput").ap()
    wq = nc.dram_tensor("wq", [D, FB], BF, kind="ExternalInput").ap()
    wk = nc.dram_tensor("wk", [D, FB], BF, kind="ExternalInput").ap()
    wv = nc.dram_tensor("wv", [D, FB], BF, kind="ExternalInput").ap()
    wo = nc.dram_tensor("wo", [FB, D], BF, kind="ExternalInput").ap()
    validT = nc.dram_tensor("validT", [nvb, BLK, BLK], BF, kind="ExternalInput").ap()
    out = nc.dram_tensor("out", [S, D], F32, kind="ExternalOutput").ap()

    # chunk coverage per i-block: for each chunk, the ordered list of
    # plan-entry indices whose [c0, c1) covers it.
    chunk_es = []
    for ib in range(NIB):
        jts = plan[ib]
        per_chunk = []
        for ic in range(NCH):
            es = [e for e, (jt, c0, c1, mk) in enumerate(jts)
                  if c0 <= ic * BLK < c1]
            per_chunk.append(es)
        chunk_es.append(per_chunk)

    with tile.TileContext(nc) as tc:
        import contextlib

        ctxmgr = contextlib.ExitStack()
        with ctxmgr:
            persist = ctxmgr.enter_context(tc.tile_pool(name="persist", bufs=1))
            xvp = ctxmgr.enter_context(tc.tile_pool(name="xvp", bufs=4))
            projp = ctxmgr.enter_context(tc.tile_pool(name="projp", bufs=1, space="PSUM"))
            scp = ctxmgr.enter_context(tc.tile_pool(name="scp", bufs=2, space="PSUM"))
            ctxp = ctxmgr.enter_context(tc.tile_pool(name="ctxp", bufs=3, space="PSUM"))
            atp = ctxmgr.enter_context(tc.tile_pool(name="atp", bufs=3))
            normp = ctxmgr.enter_context(tc.tile_pool(name="normp", bufs=6))
            rcpp = ctxmgr.enter_context(tc.tile_pool(name="rcpp", bufs=3))
            rawp = ctxmgr.enter_context(tc.tile_pool(name="rawp", bufs=4))
            outp = ctxmgr.enter_context(tc.tile_pool(name="outp", bufs=2))

            # ---- persistent tiles ----------------------------------------
            wk_all = persist.tile([BLK, NDT * FB], BF, name="wk_all")
            wq_all = persist.tile([BLK, NDT * FB], BF, name="wq_all")
            wv_all = persist.tile([BLK, NDT * FB], BF, name="wv_all")
            wo_all = persist.tile([BLK, NPAIR * D], BF, name="wo_all")
            xk_t = [persist.tile([BLK, NDT * IBW], BF, name=f"xk{ib}")
                    for ib in range(NIB)]
            xq_t = [persist.tile([BLK, NDT * IBW], BF, name=f"xq{ib}")
                    for ib in range(NIB)]
            valid_sb = persist.tile([BLK, nvb * BLK], BF, name="valid_sb")
            kT_sb = [persist.tile([BLK, S], BF, name=f"kT{p}") for p in range(NPAIR)]
            qT_sb = [persist.tile([BLK, S], BF, name=f"qT{p}") for p in range(NPAIR)]
            v_sb = [persist.tile([BLK, HPG * VW], BF, name=f"v{j}") for j in range(NJT)]
            ctxT_sb = [persist.tile([BLK, S], BF, name=f"cT{p}") for p in range(NPAIR)]

            # ---- input DMAs (consolidated; queue order sets arbitration) -
            xv_pairs = [
                xvp.tile([BLK, 2 * NDT * BLK], BF, tag="xv", name=f"xvp{jp}")
                for jp in range(NJT // 2)
            ]

            def load_x(t, src_ap, ib):
                nc.sync.dma_start(
                    out=t[ib].rearrange("p (d s) -> p d s", d=NDT),
                    in_=src_ap[:, ib * IBW:(ib + 1) * IBW].rearrange(
                        "(d p) s -> p d s", p=BLK))

            def load_xv(jp):
                nc.sync.dma_start(
                    out=xv_pairs[jp].rearrange("p (j c w) -> p j c w", j=2, c=NDT),
                    in_=xvS[2 * jp:2 * jp + 2].rearrange(
                        "j (c p) w -> p j c w", p=BLK),
                )

            # sync carries the latency-critical loads; scalar only weights so
            # the exp queue is never blocked by a waiting DMA issue.
            # The very first d-chunk of xk/wk is loaded separately so the
            # first projection matmuls start ~2us in.
            nc.sync.dma_start(out=xk_t[0][:, 0:IBW], in_=xkT[0:BLK, 0:IBW])
            nc.scalar.dma_start(out=wk_all[:, 0:FB], in_=wk[0:BLK, :])
            nc.sync.dma_start(
                out=xk_t[0].rearrange("p (d s) -> p d s", d=NDT)[:, 1:, :],
                in_=xkT[BLK:D, 0:IBW].rearrange("(d p) s -> p d s", p=BLK))
            nc.scalar.dma_start(
                out=wk_all.rearrange("p (d f) -> p d f", d=NDT)[:, 1:, :],
                in_=wk[BLK:D, :].rearrange("(d p) f -> p d f", p=BLK))
            load_x(xq_t, xqT, 0)
            nc.scalar.dma_start(
                out=wq_all.rearrange("p (d f) -> p d f", d=NDT),
                in_=wq.rearrange("(d p) f -> p d f", p=BLK))
            nc.sync.dma_start(
                out=wv_all.rearrange("p (d f) -> p d f", d=NDT),
                in_=wv.rearrange("(d p) f -> p d f", p=BLK))
            nc.scalar.dma_start(
                out=valid_sb.rearrange("p (n w) -> p n w", n=nvb),
                in_=validT.rearrange("n p w -> p n w"))
            load_xv(0)
            load_xv(1)
            load_x(xk_t, xkT, 1)
            load_x(xq_t, xqT, 1)
            nc.scalar.dma_start(
                out=wo_all.rearrange("p (g f) -> p g f", g=NPAIR),
                in_=wo.rearrange("(g p) f -> p g f", p=BLK))
            load_xv(2)
            load_xv(3)
            load_x(xk_t, xkT, 3)
            load_x(xq_t, xqT, 3)
            load_xv(4)
            load_xv(5)
            load_xv(6)
            load_xv(7)
            load_x(xk_t, xkT, 2)
            load_x(xq_t, xqT, 2)

            # ---- filler unit generators ----------------------------------
            def v_unit(jt):
                ps = projp.tile([BLK, FB], F32, tag="pp", name=f"vps{jt}")
                pt = xv_pairs[jt // 2]
                off = (jt % 2) * NDT * BLK
                for d in range(NDT):
                    def mm(d=d):
                        nc.tensor.matmul(
                            ps,
                            pt[:, off + d * BLK:off + (d + 1) * BLK],
                            wv_all[:, d * FB:(d + 1) * FB],
                            start=(d == 0),
                            stop=(d == NDT - 1),
                        )
                    yield (MM512, mm)

                def evac():
                    dst = v_sb[jt].rearrange("p (h w) -> p h w", w=VW)
                    nc.vector.tensor_copy(
                        dst[:, :, 0:HD], ps.rearrange("p (h w) -> p h w", w=HD))
                    nc.gpsimd.memset(dst[:, :, HD:VW], 1.0)
                yield (0.0, evac)

            def kq_unit(kind, p, ib):
                w_all = wk_all if kind == "k" else wq_all
                xt = (xk_t if kind == "k" else xq_t)[ib]
                dst = (kT_sb if kind == "k" else qT_sb)[p]
                ps = projp.tile([BLK, IBW], F32, tag="pp", name=f"{kind}ps{p}_{ib}")
                for d in range(NDT):
                    rhs = xt[:, d * IBW:(d + 1) * IBW]

                    def mm(d=d, rhs=rhs):
                        nc.tensor.matmul(
                            ps,
                            w_all[:, d * FB + p * BLK:d * FB + (p + 1) * BLK],
                            rhs,
                            start=(d == 0),
                            stop=(d == NDT - 1),
                        )
                    yield (MM512, mm)

                def evac():
                    nc.vector.tensor_copy(dst[:, ib * IBW:(ib + 1) * IBW], ps)
                yield (0.0, evac)

            pending_dmas = []

            def flush_dmas():
                while pending_dmas:
                    pending_dmas.pop(0)()

            def out_unit(st, nb):
                # issue the previous unit's store first - its data is ready
                # by now, so the DMA never parks on the SP queue waiting.
                yield (0.0, flush_dmas)
                po = projp.tile([BLK, IBW], F32, tag="pp", name=f"po{st}_{nb}")
                for p in range(NPAIR):
                    def mm(p=p):
                        nc.tensor.matmul(
                            po,
                            ctxT_sb[p][:, st * BLK:(st + 1) * BLK],
                            wo_all[:, p * D + nb * IBW:p * D + (nb + 1) * IBW],
                            start=(p == 0),
                            stop=(p == NPAIR - 1),
                        )
                    yield (MM512, mm)

                def store():
                    ot = outp.tile([BLK, IBW], F32, tag="ot", name=f"ot{st}_{nb}")
                    nc.vector.tensor_copy(ot, po)
                    pending_dmas.append(lambda ot=ot: nc.sync.dma_start(
                        out=out[st * BLK:(st + 1) * BLK, nb * IBW:(nb + 1) * IBW],
                        in_=ot,
                    ))
                yield (0.0, store)

            IB_ORDER = [0, 1, 3, 2]
            feeder = Feeder()
            v_added = set()
            k_added = set()
            for ib in IB_ORDER:
                for p in range(NPAIR):
                    # att(ib) reads kT columns for every j-tile <= 4*ib+3,
                    # i.e. k-projections of ALL column-blocks <= ib.
                    for jb in range(ib + 1):
                        if (p, jb) not in k_added:
                            k_added.add((p, jb))
                            feeder.add(f"k{p}_{jb}", kq_unit("k", p, jb))
                    feeder.add(f"q{p}_{ib}", kq_unit("q", p, ib))
                for jt in range(4 * ib + 4):
                    if jt not in v_added:
                        v_added.add(jt)
                        feeder.add(f"v{jt}", v_unit(jt))

            # ---- attention slots (i-block major) -------------------------
            pending_tp = []

            def flush_tp():
                while pending_tp:
                    pending_tp.pop(0)()

            # fill the PE queue with projection units before the first
            # exp-gated slot so early DMA latency is hidden.
            feeder.spend(9000.0, cap=9000.0)
            for ib in IB_ORDER:
                jts = plan[ib]
                nj = len(jts)
                ces = chunk_es[ib]
                for p in range(NPAIR):
                    flush_tp()
                    for jb in range(ib + 1):
                        feeder.drain_until(f"k{p}_{jb}")
                    feeder.drain_until(f"q{p}_{ib}")

                    ctA = ctxp.tile([BLK, 2 * 2 * VW], F32, tag="ctx",
                                    name=f"cA{p}_{ib}")
                    ctB = ctxp.tile([BLK, 2 * 2 * VW], F32, tag="ctx",
                                    name=f"cB{p}_{ib}")
                    ctx_t = (ctA, ctA, ctB, ctB)
                    # one PSUM accumulation group per tile (= per 2KB zero
                    # region): start on the first matmul emitted into it,
                    # stop on the last.
                    group_total = [0, 0]
                    for ic in range(NCH):
                        group_total[ic // 2] += 2 * len(ces[ic])
                    group_count = [0, 0]

                    sc_t = {}

                    def emit_scores(e, p=p, ib=ib, jts=jts, sc_t=sc_t):
                        jt, c0, c1, mixed = jts[e]
                        sc = scp.tile([BLK, 2 * IBW], F32, tag="sc",
                                      name=f"s{p}_{ib}_{jt}")
                        nc.tensor.matmul(
                            sc[:, c0:c1],
                            kT_sb[p][0:HD, jt * BLK:(jt + 1) * BLK],
                            qT_sb[p][0:HD, ib * IBW + c0:ib * IBW + c1],
                            start=True,
                            stop=True,
                        )
                        nc.tensor.matmul(
                            sc[:, IBW:IBW + (c1 - c0)],
                            kT_sb[p][HD:BLK, jt * BLK:(jt + 1) * BLK],
                            qT_sb[p][HD:BLK, ib * IBW + c0:ib * IBW + c1],
                            start=True,
                            stop=True,
                            tile_position=(HD, 0),
                        )
                        sc_t[e] = sc

                    def emit_tail(e, p=p, ib=ib, jts=jts, sc_t=sc_t,
                                  ctx_t=ctx_t, ces=ces):
                        jt, c0, c1, mixed = jts[e]
                        w = c1 - c0
                        sc = sc_t.pop(e)
                        at = atp.tile([BLK, 2 * IBW], BF, tag="at",
                                      name=f"a{p}_{ib}_{jt}")
                        nc.scalar.activation(
                            out=at[:, c0:IBW + w],
                            in_=sc[:, c0:IBW + w],
                            func=mybir.ActivationFunctionType.Exp,
                            scale=float(EXP_SCALE),
                        )
                        for k in mixed:
                            slot = mixed_slots[(jt, 4 * ib + k)]
                            vs = valid_sb[:, slot * BLK:(slot + 1) * BLK]
                            nc.vector.tensor_mul(
                                at[:, k * BLK:(k + 1) * BLK],
                                at[:, k * BLK:(k + 1) * BLK],
                                vs,
                            )
                            h1c = IBW + k * BLK - c0
                            nc.vector.tensor_mul(
                                at[:, h1c:h1c + BLK], at[:, h1c:h1c + BLK], vs
                            )
                        vv = v_sb[jt].rearrange("p (h w) -> p h w", w=VW)
                        for ic in range(c0 // BLK, c1 // BLK):
                            ct = ctx_t[ic]
                            gi = ic // 2
                            g0 = 2 * (ic % 2)
                            for h in range(2):
                                if h == 0:
                                    lh = at[:, ic * BLK:(ic + 1) * BLK]
                                else:
                                    base = IBW + ic * BLK - c0
                                    lh = at[:, base:base + BLK]
                                nc.tensor.matmul(
                                    ct[:, (g0 + h) * VW:(g0 + h + 1) * VW],
                                    lh,
                                    vv[:, 2 * p + h, :],
                                    start=(group_count[gi] == 0),
                                    stop=(group_count[gi] == group_total[gi] - 1),
                                )
                                group_count[gi] += 1

                    emit_scores(0)
                    for e in range(nj):
                        jt, c0, c1, mixed = jts[e]
                        w = c1 - c0
                        if e + 1 < nj:
                            emit_scores(e + 1)
                            nc0, nc1 = jts[e + 1][1], jts[e + 1][2]
                            pe_next = 2 * (nc1 - nc0) * PE_NS_PER_COL
                        else:
                            pe_next = 0.0
                        nch_e = (c1 - c0) // BLK
                        act_e = (IBW + w - c0) * ACT_NS_PER_COL + ACT_FIXED
                        pe_e = pe_next + nch_e * 2 * VW * PE_NS_PER_COL
                        bonus = 1500.0 if e == 0 else 0.0
                        feeder.spend(act_e - pe_e + bonus)
                        feeder.drain_until(f"v{jt}")
                        emit_tail(e)

                    # evacuate raw ctx PSUM to SBUF immediately (frees the
                    # bank for the next slot's group), then normalize +
                    # xbar-transpose into pair-stacked ctxT off the critical
                    # path.
                    for tix, ct in ((0, ctA), (1, ctB)):
                        ctf = rawp.tile([BLK, 2 * 2 * VW], F32, tag="ctf",
                                        name=f"ctf{p}_{ib}_{tix}")
                        nc.vector.tensor_copy(ctf, ct)
                        rcp = rcpp.tile([BLK, 4], F32, tag="rcp",
                                        name=f"rc{p}_{ib}_{tix}")
                        dens = ctf.rearrange("p (c x) -> p c x", x=VW)[:, :, HD]
                        nc.vector.reciprocal(out=rcp, in_=dens)
                        for icl in range(2):
                            ic = 2 * tix + icl
                            cn = normp.tile([BLK, 2 * HD], BF, tag="cn",
                                            name=f"cn{p}_{ib}_{ic}")
                            for h in range(2):
                                g = 2 * icl + h
                                nc.vector.tensor_scalar_mul(
                                    cn[:, h * HD:(h + 1) * HD],
                                    ctf[:, g * VW:g * VW + HD],
                                    rcp[:, g:g + 1],
                                )
                            pending_tp.append(
                                lambda p=p, ib=ib, ic=ic, cn=cn:
                                nc.sync.dma_start_transpose(
                                    out=ctxT_sb[p][:, ib * IBW + ic * BLK:
                                                   ib * IBW + (ic + 1) * BLK],
                                    in_=cn,
                                ))

                # output projection for this i-block becomes filler work
                for st in range(4 * ib, 4 * ib + 4):
                    for nb in range(2):
                        feeder.add(f"o{st}_{nb}", out_unit(st, nb))

            flush_tp()
            feeder.drain_all()
            flush_dmas()

    nc.compile()
    return nc


_BUILD_CACHE: dict = {}


def _get_nc(mask: np.ndarray):
    states = classify_mask(mask)
    plan, mixed_slots = build_plan(states)
    key = plan_key(plan, mixed_slots)
    if key not in _BUILD_CACHE:
        _BUILD_CACHE[key] = (build_nc(plan, mixed_slots), plan, mixed_slots)
    return _BUILD_CACHE[key]


def _make_in_maps(xq, xk, xv, mask, W_q, W_k, W_v, W_o, mixed_slots):
    nvb = max(1, len(mixed_slots))
    vt = np.zeros((nvb, BLK, BLK), BF16)
    m = np.asarray(mask)
    for (jt, it), slot in mixed_slots.items():
        vt[slot] = (~m[it * BLK:(it + 1) * BLK, jt * BLK:(jt + 1) * BLK]).T.astype(BF16)
    xT = {}
    for b in range(B):
        xqTb = np.asarray(xq[b]).T.astype(BF16)
        xkTb = np.asarray(xk[b]).T.astype(BF16)
        xvTb = np.asarray(xv[b]).T.astype(BF16)
        # slab layout: [jt, (d-chunk, partition), j-col]
        xvSb = np.ascontiguousarray(
            xvTb.reshape(NDT, BLK, NJT, BLK).transpose(2, 0, 1, 3)
        ).reshape(NJT, NDT * BLK, BLK)
        xT[b] = (xqTb, xkTb, xvSb)
    in_maps = []
    for c in range(8):
        b, g = c // G, c % G
        cols = slice(g * FB, (g + 1) * FB)
        in_maps.append(
            {
                "xqT": xT[b][0],
                "xkT": xT[b][1],
                "xvS": xT[b][2],
                "wq": np.asarray(W_q)[:, cols].astype(BF16),
                "wk": np.asarray(W_k)[:, cols].astype(BF16),
                "wv": np.asarray(W_v)[:, cols].astype(BF16),
                "wo": np.asarray(W_o)[cols, :].astype(BF16),
                "validT": vt,
            }
        )
    return in_maps


PROFILE = False
last_hw_exec_ns = None


def kernel(xq, xk, xv, mask, W_q, W_k, W_v, W_o):
    global last_hw_exec_ns
    from concourse import bass_utils

    nc, plan, mixed_slots = _get_nc(mask)
    in_maps = _make_in_maps(xq, xk, xv, mask, W_q, W_k, W_v, W_o, mixed_slots)
    res = bass_utils.run_bass_kernel_spmd(
        nc, in_maps, core_ids=list(range(8))
    )
    if res.exec_time_ns:
        last_hw_exec_ns = res.exec_time_ns
    out = np.empty((B, S, D), np.float32)
    for b in range(B):
        out[b] = res.results[2 * b]["out"] + res.results[2 * b + 1]["out"]
    return out


# revision 24
# speedup vs baseline: 1.0625x; 1.0625x over previous
"""Multi-head attention (B=4, S=2048, D=1024, H=16, HD=64) on 8 TRN2 NeuronCores.

Sharding: core c handles batch b=c//2 and head-group g=c%2 (8 heads).
W_q/W_k/W_v column-sharded, W_o row-sharded; the two partial outputs per
batch are summed on the host.

Per-core kernel (matmuls bf16, accumulation fp32 in PSUM):
  Projections: x^T [D, S] bf16 tiles so TensorE contracts over D directly.
  qT/kT [128, S] per head-pair (two heads stacked on partitions);
  v [S, 8*65] with a ones column per head.

  Attention in transposed layout: scoresT[j, i] = k q^T via two K=64
  matmuls per j-tile (tile_position stacks the pair on the PE array);
  exp on ScalarE (scale=1/8 folded in); causal structure from the mask at
  build time (fully-masked 128x128 blocks skipped, mixed blocks zeroed by
  a 0/1 valid matrix).

  ctx is computed in the flipped orientation: lhsT = attn tile [j, i-chunk],
  rhs = [v_h | 1] [j, 65] streaming only 65 columns per (head, j-tile,
  i-chunk) at K=128 - half the PE cost of streaming the i range at M=65.
  The ones column makes PSUM col 64 the softmax denominator, which is now a
  per-partition scalar: one DVE reciprocal + tensor_scalar multiply
  normalizes while evacuating (no DRAM bounce).  A DMA xbar transpose
  ([i, (h,f)] -> [(h,f), i]) builds the pair-stacked ctxT for the output
  projection without touching PE.

  Emission is i-block-major with a credit-based filler queue: projection
  and output-projection matmul units are interleaved into the exp-gated
  attention stream so the in-order PE queue always has work while ScalarE
  computes exponentials.
"""

import sys

sys.path.insert(0, "/opt/trn_rl_repo")

import numpy as np
import ml_dtypes

import concourse.bacc as bacc
import concourse.tile as tile
from concourse import mybir

BF16 = ml_dtypes.bfloat16
F32 = mybir.dt.float32
BF = mybir.dt.bfloat16

B, S, D, H, HD = 4, 2048, 1024, 16, 64
G = 2              # head groups (cores per batch)
HPG = H // G       # 8 heads per group
NPAIR = HPG // 2   # 4 head pairs
FB = HPG * HD      # 512 projection cols per group
BLK = 128          # j-tile size
IBW = 512          # i-block width
NIB = S // IBW     # 4 i-blocks
NJT = S // BLK     # 16 j-tiles
NDT = D // BLK     # 8 contraction tiles
NST = S // BLK     # 16 s-tiles for the output projection
VW = HD + 1        # 65: v plus ones column
NCH = IBW // BLK   # 4 i-chunks per i-block
EXP_SCALE = 1.0 / np.sqrt(np.float32(HD))

# cost-model constants for the filler credit heuristic (ns)
PE_NS_PER_COL = 1.0 / 2.4
ACT_NS_PER_COL = 1.0 / 1.2
ACT_FIXED = 185.0
MM512 = 512 * PE_NS_PER_COL


def classify_mask(mask: np.ndarray):
    """Block states over the *transposed* mask grid: state[jt][it] for the
    region j in [128jt,..), i in [128it,..).  0=all valid, 1=all masked,
    2=mixed."""
    m = np.asarray(mask)
    blocks = m.reshape(NJT, BLK, NJT, BLK).transpose(0, 2, 1, 3)  # [it, jt, i, j]
    anym = blocks.any(axis=(2, 3))
    allm = blocks.all(axis=(2, 3))
    states = np.where(allm, 1, np.where(anym, 2, 0)).astype(np.int8)
    return states.T  # index [jt, it]


def build_plan(states: np.ndarray):
    """Per i-block: list of (jt, c0, c1, mixed_ks).  c0/c1 bound the valid
    i-columns (relative to the block) at 128 granularity; mixed_ks are the
    128-col sub-blocks needing a 0/1 multiply (includes interior all-masked
    sub-blocks, which get an all-zero valid matrix)."""
    plan = []
    mixed_slots = {}
    for ib in range(NIB):
        its = list(range(4 * ib, 4 * ib + 4))
        jts = []
        for jt in range(NJT):
            sub = [int(states[jt, it]) for it in its]
            nz = [k for k, st in enumerate(sub) if st != 1]
            if not nz:
                continue
            k0, k1 = nz[0], nz[-1]
            mixed = [k for k in range(k0, k1 + 1) if sub[k] != 0]
            for k in mixed:
                mixed_slots.setdefault((jt, its[k]), len(mixed_slots))
            jts.append((jt, k0 * BLK, (k1 + 1) * BLK, mixed))
        assert jts, "fully-masked i-block not supported"
        plan.append(jts)
    return plan, mixed_slots


def plan_key(plan, mixed_slots):
    return (
        tuple(
            tuple((jt, c0, c1, tuple(mk)) for jt, c0, c1, mk in jts) for jts in plan
        ),
        tuple(sorted(mixed_slots.items())),
    )


class Feeder:
    """Ordered queue of PE filler-work generators.  Each generator yields
    (pe_cost_ns, emit_fn) items.  spend() emits items against an ACT-vs-PE
    credit; drain_until() force-emits everything up to a named unit so its
    consumers can be issued afterwards."""

    def __init__(self):
        self.queue = []
        self.credit = 0.0
        self.emitted = set()

    def add(self, name, gen):
        self.queue.append((name, gen))

    def _emit_one(self):
        while self.queue:
            name, gen = self.queue[0]
            try:
                cost, fn = next(gen)
            except StopIteration:
                self.emitted.add(name)
                self.queue.pop(0)
                continue
            fn()
            return cost
        return None

    def drain_until(self, name):
        while name not in self.emitted:
            if self._emit_one() is None:
                raise RuntimeError(f"feeder exhausted before {name}")

    def drain_all(self):
        while self._emit_one() is not None:
            pass

    def spend(self, ns, cap=3000.0):
        self.credit = min(self.credit + ns, cap)
        while self.credit > 0 and self.queue:
            cost = self._emit_one()
            if cost is None:
                return
            self.credit -= cost


def build_nc(plan, mixed_slots):
    nvb = max(1, len(mixed_slots))
    nc = bacc.Bacc("TRN2", target_bir_lowering=False, debug=False, num_devices=8)

    xqT = nc.dram_tensor("xqT", [D, S], BF, kind="ExternalInput").ap()
    xkT = nc.dram_tensor("xkT", [D, S], BF, kind="ExternalInput").ap()
    xvS = nc.dram_tensor("xvS", [NJT, NDT * BLK, BLK], BF, kind="ExternalInput").ap()
    wq = nc.dram_tensor("wq", [D, FB], BF, kind="ExternalInput").ap()
    wk = nc.dram_tensor("wk", [D, FB], BF, kind="ExternalInput").ap()
    wv = nc.dram_tensor("wv", [D, FB], BF, kind="ExternalInput").ap()
    wo = nc.dram_tensor("wo", [FB, D], BF, kind="ExternalInput").ap()
    validT = nc.dram_tensor("validT", [nvb, BLK, BLK], BF, kind="ExternalInput").ap()
    out = nc.dram_tensor("out", [S, D], F32, kind="ExternalOutput").ap()

    # chunk coverage per i-block: for each chunk, the ordered list of
    # plan-entry indices whose [c0, c1) covers it.
    chunk_es = []
    for ib in range(NIB):
        jts = plan[ib]
        per_chunk = []
        for ic in range(NCH):
            es = [e for e, (jt, c0, c1, mk) in enumerate(jts)
                  if c0 <= ic * BLK < c1]
            per_chunk.append(es)
        chunk_es.append(per_chunk)

    with tile.TileContext(nc) as tc:
        import contextlib

        ctxmgr = contextlib.ExitStack()
        with ctxmgr:
            persist = ctxmgr.enter_context(tc.tile_pool(name="persist", bufs=1))
            xvp = ctxmgr.enter_context(tc.tile_pool(name="xvp", bufs=4))
            projp = ctxmgr.enter_context(tc.tile_pool(name="projp", bufs=2, space="PSUM"))
            scp = ctxmgr.enter_context(tc.tile_pool(name="scp", bufs=2, space="PSUM"))
            ctxp = ctxmgr.enter_context(tc.tile_pool(name="ctxp", bufs=2, space="PSUM"))
            atp = ctxmgr.enter_context(tc.tile_pool(name="atp", bufs=3))
            normp = ctxmgr.enter_context(tc.tile_pool(name="normp", bufs=16))
            rcpp = ctxmgr.enter_context(tc.tile_pool(name="rcpp", bufs=3))
            rawp = ctxmgr.enter_context(tc.tile_pool(name="rawp", bufs=6))
            outp = ctxmgr.enter_context(tc.tile_pool(name="outp", bufs=2))

            # ---- persistent tiles ----------------------------------------
            wk_all = persist.tile([BLK, NDT * FB], BF, name="wk_all")
            wq_all = persist.tile([BLK, NDT * FB], BF, name="wq_all")
            wv_all = persist.tile([BLK, NDT * FB], BF, name="wv_all")
            wo_all = persist.tile([BLK, NPAIR * D], BF, name="wo_all")
            xk_t = [persist.tile([BLK, NDT * IBW], BF, name=f"xk{ib}")
                    for ib in range(NIB)]
            xq_t = [persist.tile([BLK, NDT * IBW], BF, name=f"xq{ib}")
                    for ib in range(NIB)]
            valid_sb = persist.tile([BLK, nvb * BLK], BF, name="valid_sb")
            kT_sb = [persist.tile([BLK, S], BF, name=f"kT{p}") for p in range(NPAIR)]
            qT_sb = [persist.tile([BLK, S], BF, name=f"qT{p}") for p in range(NPAIR)]
            v_sb = [persist.tile([BLK, HPG * VW], BF, name=f"v{j}") for j in range(NJT)]
            ctxT_sb = [persist.tile([BLK, S], BF, name=f"cT{p}") for p in range(NPAIR)]

            # ---- input DMAs (consolidated; queue order sets arbitration) -
            xv_pairs = [
                xvp.tile([BLK, 2 * NDT * BLK], BF, tag="xv", name=f"xvp{jp}")
                for jp in range(NJT // 2)
            ]

            def load_x(t, src_ap, ib):
                nc.sync.dma_start(
                    out=t[ib].rearrange("p (d s) -> p d s", d=NDT),
                    in_=src_ap[:, ib * IBW:(ib + 1) * IBW].rearrange(
                        "(d p) s -> p d s", p=BLK))

            def load_xv(jp):
                nc.sync.dma_start(
                    out=xv_pairs[jp].rearrange("p (j c w) -> p j c w", j=2, c=NDT),
                    in_=xvS[2 * jp:2 * jp + 2].rearrange(
                        "j (c p) w -> p j c w", p=BLK),
                )

            # sync carries the latency-critical loads; scalar only weights so
            # the exp queue is never blocked by a waiting DMA issue.
            # The very first d-chunk of xk/wk is loaded separately so the
            # first projection matmuls start ~2us in.
            nc.sync.dma_start(out=xk_t[0][:, 0:IBW], in_=xkT[0:BLK, 0:IBW])
            nc.scalar.dma_start(out=wk_all[:, 0:FB], in_=wk[0:BLK, :])
            nc.sync.dma_start(
                out=xk_t[0].rearrange("p (d s) -> p d s", d=NDT)[:, 1:, :],
                in_=xkT[BLK:D, 0:IBW].rearrange("(d p) s -> p d s", p=BLK))
            nc.scalar.dma_start(
                out=wk_all.rearrange("p (d f) -> p d f", d=NDT)[:, 1:, :],
                in_=wk[BLK:D, :].rearrange("(d p) f -> p d f", p=BLK))
            load_x(xq_t, xqT, 0)
            nc.scalar.dma_start(
                out=wq_all.rearrange("p (d f) -> p d f", d=NDT),
                in_=wq.rearrange("(d p) f -> p d f", p=BLK))
            nc.sync.dma_start(
                out=wv_all.rearrange("p (d f) -> p d f", d=NDT),
                in_=wv.rearrange("(d p) f -> p d f", p=BLK))
            nc.scalar.dma_start(
                out=valid_sb.rearrange("p (n w) -> p n w", n=nvb),
                in_=validT.rearrange("n p w -> p n w"))
            load_xv(0)
            load_xv(1)
            load_x(xk_t, xkT, 1)
            load_x(xq_t, xqT, 1)
            nc.scalar.dma_start(
                out=wo_all.rearrange("p (g f) -> p g f", g=NPAIR),
                in_=wo.rearrange("(g p) f -> p g f", p=BLK))
            load_xv(2)
            load_xv(3)
            load_x(xk_t, xkT, 3)
            load_x(xq_t, xqT, 3)
            load_xv(4)
            load_xv(5)
            load_xv(6)
            load_xv(7)
            load_x(xk_t, xkT, 2)
            load_x(xq_t, xqT, 2)

            # ---- filler unit generators ----------------------------------
            def v_unit(jt):
                ps = projp.tile([BLK, FB], F32, tag="pp", name=f"vps{jt}")
                pt = xv_pairs[jt // 2]
                off = (jt % 2) * NDT * BLK
                for d in range(NDT):
                    def mm(d=d):
                        nc.tensor.matmul(
                            ps,
                            pt[:, off + d * BLK:off + (d + 1) * BLK],
                            wv_all[:, d * FB:(d + 1) * FB],
                            start=(d == 0),
                            stop=(d == NDT - 1),
                        )
                    yield (MM512, mm)

                def evac():
                    dst = v_sb[jt].rearrange("p (h w) -> p h w", w=VW)
                    nc.vector.tensor_copy(
                        dst[:, :, 0:HD], ps.rearrange("p (h w) -> p h w", w=HD))
                    nc.gpsimd.memset(dst[:, :, HD:VW], 1.0)
                yield (0.0, evac)

            def kq_unit(kind, p, ib):
                w_all = wk_all if kind == "k" else wq_all
                xt = (xk_t if kind == "k" else xq_t)[ib]
                dst = (kT_sb if kind == "k" else qT_sb)[p]
                ps = projp.tile([BLK, IBW], F32, tag="pp", name=f"{kind}ps{p}_{ib}")
                for d in range(NDT):
                    rhs = xt[:, d * IBW:(d + 1) * IBW]

                    def mm(d=d, rhs=rhs):
                        nc.tensor.matmul(
                            ps,
                            w_all[:, d * FB + p * BLK:d * FB + (p + 1) * BLK],
                            rhs,
                            start=(d == 0),
                            stop=(d == NDT - 1),
                        )
                    yield (MM512, mm)

                def evac():
                    nc.vector.tensor_copy(dst[:, ib * IBW:(ib + 1) * IBW], ps)
                yield (0.0, evac)

            pending_dmas = []

            def flush_dmas():
                while pending_dmas:
                    pending_dmas.pop(0)()

            def out_unit(st, nb):
                # issue the previous unit's store first - its data is ready
                # by now, so the DMA never parks on the SP queue waiting.
                yield (0.0, flush_dmas)
                po = projp.tile([BLK, IBW], F32, tag="pp", name=f"po{st}_{nb}")
                for p in range(NPAIR):
                    def mm(p=p):
                        nc.tensor.matmul(
                            po,
                            ctxT_sb[p][:, st * BLK:(st + 1) * BLK],
                            wo_all[:, p * D + nb * IBW:p * D + (nb + 1) * IBW],
                            start=(p == 0),
                            stop=(p == NPAIR - 1),
                        )
                    yield (MM512, mm)

                def store():
                    ot = outp.tile([BLK, IBW], F32, tag="ot", name=f"ot{st}_{nb}")
                    nc.vector.tensor_copy(ot, po)
                    pending_dmas.append(lambda ot=ot: nc.scalar.dma_start(
                        out=out[st * BLK:(st + 1) * BLK, nb * IBW:(nb + 1) * IBW],
                        in_=ot,
                    ))
                yield (0.0, store)

            IB_ORDER = [0, 1, 3, 2]
            feeder = Feeder()
            v_added = set()
            k_added = set()
            for ib in IB_ORDER:
                for p in range(NPAIR):
                    # att(ib) reads kT columns for every j-tile <= 4*ib+3,
                    # i.e. k-projections of ALL column-blocks <= ib.
                    for jb in range(ib + 1):
                        if (p, jb) not in k_added:
                            k_added.add((p, jb))
                            feeder.add(f"k{p}_{jb}", kq_unit("k", p, jb))
                    feeder.add(f"q{p}_{ib}", kq_unit("q", p, ib))
                for jt in range(4 * ib + 4):
                    if jt not in v_added:
                        v_added.add(jt)
                        feeder.add(f"v{jt}", v_unit(jt))

            # ---- attention slots (i-block major) -------------------------
            pending_tp = []

            def flush_tp():
                while pending_tp:
                    pending_tp.pop(0)()

            # fill the PE queue with projection units before the first
            # exp-gated slot so early DMA latency is hidden.
            feeder.spend(9000.0, cap=9000.0)
            for ib in IB_ORDER:
                jts = plan[ib]
                nj = len(jts)
                ces = chunk_es[ib]
                for p in range(NPAIR):
                    flush_tp()
                    for jb in range(ib + 1):
                        feeder.drain_until(f"k{p}_{jb}")
                    feeder.drain_until(f"q{p}_{ib}")

                    ctA = ctxp.tile([BLK, 2 * 2 * VW], F32, tag="ctx",
                                    name=f"cA{p}_{ib}")
                    ctB = ctxp.tile([BLK, 2 * 2 * VW], F32, tag="ctx",
                                    name=f"cB{p}_{ib}")
                    ctx_t = (ctA, ctA, ctB, ctB)
                    # one PSUM accumulation group per tile (= per 2KB zero
                    # region): start on the first matmul emitted into it,
                    # stop on the last.
                    group_total = [0, 0]
                    for ic in range(NCH):
                        group_total[ic // 2] += 2 * len(ces[ic])
                    group_count = [0, 0]

                    sc_t = {}

                    def emit_scores(e, p=p, ib=ib, jts=jts, sc_t=sc_t):
                        jt, c0, c1, mixed = jts[e]
                        sc = scp.tile([BLK, 2 * IBW], F32, tag="sc",
                                      name=f"s{p}_{ib}_{jt}")
                        nc.tensor.matmul(
                            sc[:, c0:c1],
                            kT_sb[p][0:HD, jt * BLK:(jt + 1) * BLK],
                            qT_sb[p][0:HD, ib * IBW + c0:ib * IBW + c1],
                            start=True,
                            stop=True,
                        )
                        nc.tensor.matmul(
                            sc[:, IBW:IBW + (c1 - c0)],
                            kT_sb[p][HD:BLK, jt * BLK:(jt + 1) * BLK],
                            qT_sb[p][HD:BLK, ib * IBW + c0:ib * IBW + c1],
                            start=True,
                            stop=True,
                            tile_position=(HD, 0),
                        )
                        sc_t[e] = sc

                    def emit_tail(e, p=p, ib=ib, jts=jts, sc_t=sc_t,
                                  ctx_t=ctx_t, ces=ces):
                        jt, c0, c1, mixed = jts[e]
                        w = c1 - c0
                        sc = sc_t.pop(e)
                        at = atp.tile([BLK, 2 * IBW], BF, tag="at",
                                      name=f"a{p}_{ib}_{jt}")
                        nc.scalar.activation(
                            out=at[:, c0:IBW + w],
                            in_=sc[:, c0:IBW + w],
                            func=mybir.ActivationFunctionType.Exp,
                            scale=float(EXP_SCALE),
                        )
                        for k in mixed:
                            slot = mixed_slots[(jt, 4 * ib + k)]
                            vs = valid_sb[:, slot * BLK:(slot + 1) * BLK]
                            nc.vector.tensor_mul(
                                at[:, k * BLK:(k + 1) * BLK],
                                at[:, k * BLK:(k + 1) * BLK],
                                vs,
                            )
                            h1c = IBW + k * BLK - c0
                            nc.vector.tensor_mul(
                                at[:, h1c:h1c + BLK], at[:, h1c:h1c + BLK], vs
                            )
                        vv = v_sb[jt].rearrange("p (h w) -> p h w", w=VW)
                        for ic in range(c0 // BLK, c1 // BLK):
                            ct = ctx_t[ic]
                            gi = ic // 2
                            g0 = 2 * (ic % 2)
                            for h in range(2):
                                if h == 0:
                                    lh = at[:, ic * BLK:(ic + 1) * BLK]
                                else:
                                    base = IBW + ic * BLK - c0
                                    lh = at[:, base:base + BLK]
                                nc.tensor.matmul(
                                    ct[:, (g0 + h) * VW:(g0 + h + 1) * VW],
                                    lh,
                                    vv[:, 2 * p + h, :],
                                    start=(group_count[gi] == 0),
                                    stop=(group_count[gi] == group_total[gi] - 1),
                                )
                                group_count[gi] += 1

                    emit_scores(0)
                    for e in range(nj):
                        jt, c0, c1, mixed = jts[e]
                        w = c1 - c0
                        if e + 1 < nj:
                            emit_scores(e + 1)
                            nc0, nc1 = jts[e + 1][1], jts[e + 1][2]
                            pe_next = 2 * (nc1 - nc0) * PE_NS_PER_COL
                        else:
                            pe_next = 0.0
                        nch_e = (c1 - c0) // BLK
                        act_e = (IBW + w - c0) * ACT_NS_PER_COL + ACT_FIXED
                        pe_e = pe_next + nch_e * 2 * VW * PE_NS_PER_COL
                        bonus = 1500.0 if e == 0 else 0.0
                        feeder.spend(act_e - pe_e + bonus)
                        feeder.drain_until(f"v{jt}")
                        emit_tail(e)

                    # evacuate raw ctx PSUM to SBUF immediately (frees the
                    # bank for the next slot's group), then normalize +
                    # xbar-transpose into pair-stacked ctxT off the critical
                    # path.
                    for tix, ct in ((0, ctA), (1, ctB)):
                        ctf = rawp.tile([BLK, 2 * 2 * VW], F32, tag="ctf",
                                        name=f"ctf{p}_{ib}_{tix}")
                        nc.vector.tensor_copy(ctf, ct)
                        rcp = rcpp.tile([BLK, 4], F32, tag="rcp",
                                        name=f"rc{p}_{ib}_{tix}")
                        dens = ctf.rearrange("p (c x) -> p c x", x=VW)[:, :, HD]
                        nc.vector.reciprocal(out=rcp, in_=dens)
                        for icl in range(2):
                            ic = 2 * tix + icl
                            cn = normp.tile([BLK, 2 * HD], BF, tag="cn",
                                            name=f"cn{p}_{ib}_{ic}")
                            for h in range(2):
                                g = 2 * icl + h
                                nc.vector.tensor_scalar_mul(
                                    cn[:, h * HD:(h + 1) * HD],
                                    ctf[:, g * VW:g * VW + HD],
                                    rcp[:, g:g + 1],
                                )
                            pending_tp.append(
                                lambda p=p, ib=ib, ic=ic, cn=cn:
                                nc.sync.dma_start_transpose(
                                    out=ctxT_sb[p][:, ib * IBW + ic * BLK:
                                                   ib * IBW + (ic + 1) * BLK],
                                    in_=cn,
                                ))

                # output projection for this i-block becomes filler work
                for st in range(4 * ib, 4 * ib + 4):
                    for nb in range(2):
                        feeder.add(f"o{st}_{nb}", out_unit(st, nb))

            flush_tp()
            feeder.drain_all()
            flush_dmas()

    nc.compile()
    return nc


_BUILD_CACHE: dict = {}


def _get_nc(mask: np.ndarray):
    states = classify_mask(mask)
    plan, mixed_slots = build_plan(states)
    key = plan_key(plan, mixed_slots)
    if key not in _BUILD_CACHE:
        _BUILD_CACHE[key] = (build_nc(plan, mixed_slots), plan, mixed_slots)
    return _BUILD_CACHE[key]


def _make_in_maps(xq, xk, xv, mask, W_q, W_k, W_v, W_o, mixed_slots):
    nvb = max(1, len(mixed_slots))
    vt = np.zeros((nvb, BLK, BLK), BF16)
    m = np.asarray(mask)
    for (jt, it), slot in mixed_slots.items():
        vt[slot] = (~m[it * BLK:(it + 1) * BLK, jt * BLK:(jt + 1) * BLK]).T.astype(BF16)
    xT = {}
    for b in range(B):
        xqTb = np.asarray(xq[b]).T.astype(BF16)
        xkTb = np.asarray(xk[b]).T.astype(BF16)
        xvTb = np.asarray(xv[b]).T.astype(BF16)
        # slab layout: [jt, (d-chunk, partition), j-col]
        xvSb = np.ascontiguousarray(
            xvTb.reshape(NDT, BLK, NJT, BLK).transpose(2, 0, 1, 3)
        ).reshape(NJT, NDT * BLK, BLK)
        xT[b] = (xqTb, xkTb, xvSb)
    in_maps = []
    for c in range(8):
        b, g = c // G, c % G
        cols = slice(g * FB, (g + 1) * FB)
        in_maps.append(
            {
                "xqT": xT[b][0],
                "xkT": xT[b][1],
                "xvS": xT[b][2],
                "wq": np.asarray(W_q)[:, cols].astype(BF16),
                "wk": np.asarray(W_k)[:, cols].astype(BF16),
                "wv": np.asarray(W_v)[:, cols].astype(BF16),
                "wo": np.asarray(W_o)[cols, :].astype(BF16),
                "validT": vt,
            }
        )
    return in_maps


PROFILE = False
last_hw_exec_ns = None


def kernel(xq, xk, xv, mask, W_q, W_k, W_v, W_o):
    global last_hw_exec_ns
    from concourse import bass_utils

    nc, plan, mixed_slots = _get_nc(mask)
    in_maps = _make_in_maps(xq, xk, xv, mask, W_q, W_k, W_v, W_o, mixed_slots)
    res = bass_utils.run_bass_kernel_spmd(
        nc, in_maps, core_ids=list(range(8))
    )
    if res.exec_time_ns:
        last_hw_exec_ns = res.exec_time_ns
    out = np.empty((B, S, D), np.float32)
    for b in range(B):
        out[b] = res.results[2 * b]["out"] + res.results[2 * b + 1]["out"]
    return out


# revision 25
# speedup vs baseline: 1.1038x; 1.0389x over previous
"""Multi-head attention (B=4, S=2048, D=1024, H=16, HD=64) on 8 TRN2 NeuronCores.

Sharding: core c handles batch b=c//2 and head-group g=c%2 (8 heads).
W_q/W_k/W_v column-sharded, W_o row-sharded; the two partial outputs per
batch are summed on the host.

Per-core kernel (matmuls bf16, accumulation fp32 in PSUM):
  Projections: x^T [D, S] bf16 tiles so TensorE contracts over D directly.
  qT/kT [128, S] per head-pair (two heads stacked on partitions);
  v [S, 8*65] with a ones column per head.

  Attention in transposed layout: scoresT[j, i] = k q^T via two K=64
  matmuls per j-tile (tile_position stacks the pair on the PE array);
  exp on ScalarE (scale=1/8 folded in); causal structure from the mask at
  build time (fully-masked 128x128 blocks skipped, mixed blocks zeroed by
  a 0/1 valid matrix).

  ctx is computed in the flipped orientation: lhsT = attn tile [j, i-chunk],
  rhs = [v_h | 1] [j, 65] streaming only 65 columns per (head, j-tile,
  i-chunk) at K=128 - half the PE cost of streaming the i range at M=65.
  The ones column makes PSUM col 64 the softmax denominator, which is now a
  per-partition scalar: one DVE reciprocal + tensor_scalar multiply
  normalizes while evacuating (no DRAM bounce).  A DMA xbar transpose
  ([i, (h,f)] -> [(h,f), i]) builds the pair-stacked ctxT for the output
  projection without touching PE.

  Emission is i-block-major with a credit-based filler queue: projection
  and output-projection matmul units are interleaved into the exp-gated
  attention stream so the in-order PE queue always has work while ScalarE
  computes exponentials.
"""

import sys

sys.path.insert(0, "/opt/trn_rl_repo")

import numpy as np
import ml_dtypes

import concourse.bacc as bacc
import concourse.tile as tile
from concourse import mybir

BF16 = ml_dtypes.bfloat16
F32 = mybir.dt.float32
BF = mybir.dt.bfloat16

B, S, D, H, HD = 4, 2048, 1024, 16, 64
G = 2              # head groups (cores per batch)
HPG = H // G       # 8 heads per group
NPAIR = HPG // 2   # 4 head pairs
FB = HPG * HD      # 512 projection cols per group
BLK = 128          # j-tile size
IBW = 512          # i-block width
NIB = S // IBW     # 4 i-blocks
NJT = S // BLK     # 16 j-tiles
NDT = D // BLK     # 8 contraction tiles
NST = S // BLK     # 16 s-tiles for the output projection
VW = HD + 1        # 65: v plus ones column
NCH = IBW // BLK   # 4 i-chunks per i-block
EXP_SCALE = 1.0 / np.sqrt(np.float32(HD))

# cost-model constants for the filler credit heuristic (ns)
PE_NS_PER_COL = 1.0 / 2.4
ACT_NS_PER_COL = 1.0 / 1.2
ACT_FIXED = 185.0
MM512 = 512 * PE_NS_PER_COL


def classify_mask(mask: np.ndarray):
    """Block states over the *transposed* mask grid: state[jt][it] for the
    region j in [128jt,..), i in [128it,..).  0=all valid, 1=all masked,
    2=mixed."""
    m = np.asarray(mask)
    blocks = m.reshape(NJT, BLK, NJT, BLK).transpose(0, 2, 1, 3)  # [it, jt, i, j]
    anym = blocks.any(axis=(2, 3))
    allm = blocks.all(axis=(2, 3))
    states = np.where(allm, 1, np.where(anym, 2, 0)).astype(np.int8)
    return states.T  # index [jt, it]


def build_plan(states: np.ndarray):
    """Per i-block: list of (jt, c0, c1, mixed_ks).  c0/c1 bound the valid
    i-columns (relative to the block) at 128 granularity; mixed_ks are the
    128-col sub-blocks needing a 0/1 multiply (includes interior all-masked
    sub-blocks, which get an all-zero valid matrix)."""
    plan = []
    mixed_slots = {}
    for ib in range(NIB):
        its = list(range(4 * ib, 4 * ib + 4))
        jts = []
        for jt in range(NJT):
            sub = [int(states[jt, it]) for it in its]
            nz = [k for k, st in enumerate(sub) if st != 1]
            if not nz:
                continue
            k0, k1 = nz[0], nz[-1]
            mixed = [k for k in range(k0, k1 + 1) if sub[k] != 0]
            for k in mixed:
                mixed_slots.setdefault((jt, its[k]), len(mixed_slots))
            jts.append((jt, k0 * BLK, (k1 + 1) * BLK, mixed))
        assert jts, "fully-masked i-block not supported"
        plan.append(jts)
    return plan, mixed_slots


def plan_key(plan, mixed_slots):
    return (
        tuple(
            tuple((jt, c0, c1, tuple(mk)) for jt, c0, c1, mk in jts) for jts in plan
        ),
        tuple(sorted(mixed_slots.items())),
    )


class Feeder:
    """Ordered queue of PE filler-work generators.  Each generator yields
    (pe_cost_ns, emit_fn) items.  spend() emits items against an ACT-vs-PE
    credit; drain_until() force-emits everything up to a named unit so its
    consumers can be issued afterwards."""

    def __init__(self):
        self.queue = []
        self.credit = 0.0
        self.emitted = set()

    def add(self, name, gen):
        self.queue.append((name, gen))

    def _emit_one(self):
        while self.queue:
            name, gen = self.queue[0]
            try:
                cost, fn = next(gen)
            except StopIteration:
                self.emitted.add(name)
                self.queue.pop(0)
                continue
            fn()
            return cost
        return None

    def drain_until(self, name):
        while name not in self.emitted:
            if self._emit_one() is None:
                raise RuntimeError(f"feeder exhausted before {name}")

    def drain_all(self):
        while self._emit_one() is not None:
            pass

    def spend(self, ns, cap=3000.0):
        self.credit = min(self.credit + ns, cap)
        while self.credit > 0 and self.queue:
            cost = self._emit_one()
            if cost is None:
                return
            self.credit -= cost


def build_nc(plan, mixed_slots):
    nvb = max(1, len(mixed_slots))
    nc = bacc.Bacc("TRN2", target_bir_lowering=False, debug=False, num_devices=8)

    xqT = nc.dram_tensor("xqT", [D, S], BF, kind="ExternalInput").ap()
    xkT = nc.dram_tensor("xkT", [D, S], BF, kind="ExternalInput").ap()
    xvS = nc.dram_tensor("xvS", [NJT, NDT * BLK, BLK], BF, kind="ExternalInput").ap()
    wq = nc.dram_tensor("wq", [D, FB], BF, kind="ExternalInput").ap()
    wk = nc.dram_tensor("wk", [D, FB], BF, kind="ExternalInput").ap()
    wv = nc.dram_tensor("wv", [D, FB], BF, kind="ExternalInput").ap()
    wo = nc.dram_tensor("wo", [FB, D], BF, kind="ExternalInput").ap()
    validT = nc.dram_tensor("validT", [nvb, BLK, BLK], BF, kind="ExternalInput").ap()
    out = nc.dram_tensor("out", [S, D], F32, kind="ExternalOutput").ap()

    # chunk coverage per i-block: for each chunk, the ordered list of
    # plan-entry indices whose [c0, c1) covers it.
    chunk_es = []
    for ib in range(NIB):
        jts = plan[ib]
        per_chunk = []
        for ic in range(NCH):
            es = [e for e, (jt, c0, c1, mk) in enumerate(jts)
                  if c0 <= ic * BLK < c1]
            per_chunk.append(es)
        chunk_es.append(per_chunk)

    with tile.TileContext(nc) as tc:
        import contextlib

        ctxmgr = contextlib.ExitStack()
        with ctxmgr:
            persist = ctxmgr.enter_context(tc.tile_pool(name="persist", bufs=1))
            xvp = ctxmgr.enter_context(tc.tile_pool(name="xvp", bufs=4))
            projp = ctxmgr.enter_context(tc.tile_pool(name="projp", bufs=2, space="PSUM"))
            scp = ctxmgr.enter_context(tc.tile_pool(name="scp", bufs=2, space="PSUM"))
            ctxp = ctxmgr.enter_context(tc.tile_pool(name="ctxp", bufs=2, space="PSUM"))
            atp = ctxmgr.enter_context(tc.tile_pool(name="atp", bufs=3))
            normp = ctxmgr.enter_context(tc.tile_pool(name="normp", bufs=16))
            rcpp = ctxmgr.enter_context(tc.tile_pool(name="rcpp", bufs=3))
            rawp = ctxmgr.enter_context(tc.tile_pool(name="rawp", bufs=6))
            outp = ctxmgr.enter_context(tc.tile_pool(name="outp", bufs=2))

            # ---- persistent tiles ----------------------------------------
            wk_all = persist.tile([BLK, NDT * FB], BF, name="wk_all")
            wq_all = persist.tile([BLK, NDT * FB], BF, name="wq_all")
            wv_all = persist.tile([BLK, NDT * FB], BF, name="wv_all")
            wo_all = persist.tile([BLK, NPAIR * D], BF, name="wo_all")
            xk_t = [persist.tile([BLK, NDT * IBW], BF, name=f"xk{ib}")
                    for ib in range(NIB)]
            xq_t = [persist.tile([BLK, NDT * IBW], BF, name=f"xq{ib}")
                    for ib in range(NIB)]
            valid_sb = persist.tile([BLK, nvb * BLK], BF, name="valid_sb")
            kT_sb = [persist.tile([BLK, S], BF, name=f"kT{p}") for p in range(NPAIR)]
            qT_sb = [persist.tile([BLK, S], BF, name=f"qT{p}") for p in range(NPAIR)]
            v_sb = [persist.tile([BLK, HPG * VW], BF, name=f"v{j}") for j in range(NJT)]
            ctxT_sb = [persist.tile([BLK, S], BF, name=f"cT{p}") for p in range(NPAIR)]

            # ---- input DMAs (consolidated; queue order sets arbitration) -
            xv_pairs = [
                xvp.tile([BLK, 2 * NDT * BLK], BF, tag="xv", name=f"xvp{jp}")
                for jp in range(NJT // 2)
            ]

            def load_x(t, src_ap, ib):
                nc.sync.dma_start(
                    out=t[ib].rearrange("p (d s) -> p d s", d=NDT),
                    in_=src_ap[:, ib * IBW:(ib + 1) * IBW].rearrange(
                        "(d p) s -> p d s", p=BLK))

            def load_xv(jp):
                nc.sync.dma_start(
                    out=xv_pairs[jp].rearrange("p (j c w) -> p j c w", j=2, c=NDT),
                    in_=xvS[2 * jp:2 * jp + 2].rearrange(
                        "j (c p) w -> p j c w", p=BLK),
                )

            # sync carries the latency-critical loads; scalar only weights so
            # the exp queue is never blocked by a waiting DMA issue.
            # The very first d-chunk of xk/wk is loaded separately so the
            # first projection matmuls start ~2us in.
            nc.sync.dma_start(out=xk_t[0][:, 0:IBW], in_=xkT[0:BLK, 0:IBW])
            nc.scalar.dma_start(out=wk_all[:, 0:FB], in_=wk[0:BLK, :])
            nc.sync.dma_start(
                out=xk_t[0].rearrange("p (d s) -> p d s", d=NDT)[:, 1:, :],
                in_=xkT[BLK:D, 0:IBW].rearrange("(d p) s -> p d s", p=BLK))
            nc.scalar.dma_start(
                out=wk_all.rearrange("p (d f) -> p d f", d=NDT)[:, 1:, :],
                in_=wk[BLK:D, :].rearrange("(d p) f -> p d f", p=BLK))
            load_x(xq_t, xqT, 0)
            nc.scalar.dma_start(
                out=wq_all.rearrange("p (d f) -> p d f", d=NDT),
                in_=wq.rearrange("(d p) f -> p d f", p=BLK))
            nc.sync.dma_start(
                out=wv_all.rearrange("p (d f) -> p d f", d=NDT),
                in_=wv.rearrange("(d p) f -> p d f", p=BLK))
            nc.scalar.dma_start(
                out=valid_sb.rearrange("p (n w) -> p n w", n=nvb),
                in_=validT.rearrange("n p w -> p n w"))
            load_xv(0)
            load_xv(1)
            load_x(xk_t, xkT, 1)
            load_x(xq_t, xqT, 1)
            nc.scalar.dma_start(
                out=wo_all.rearrange("p (g f) -> p g f", g=NPAIR),
                in_=wo.rearrange("(g p) f -> p g f", p=BLK))
            load_xv(2)
            load_xv(3)
            load_x(xk_t, xkT, 3)
            load_x(xq_t, xqT, 3)
            load_xv(4)
            load_xv(5)
            load_xv(6)
            load_xv(7)
            load_x(xk_t, xkT, 2)
            load_x(xq_t, xqT, 2)

            # ---- filler unit generators ----------------------------------
            def v_unit(jt):
                ps = projp.tile([BLK, FB], F32, tag="pp", name=f"vps{jt}")
                pt = xv_pairs[jt // 2]
                off = (jt % 2) * NDT * BLK
                for d in range(NDT):
                    def mm(d=d):
                        nc.tensor.matmul(
                            ps,
                            pt[:, off + d * BLK:off + (d + 1) * BLK],
                            wv_all[:, d * FB:(d + 1) * FB],
                            start=(d == 0),
                            stop=(d == NDT - 1),
                        )
                    yield (MM512, mm)

                def evac():
                    dst = v_sb[jt].rearrange("p (h w) -> p h w", w=VW)
                    nc.vector.tensor_copy(
                        dst[:, :, 0:HD], ps.rearrange("p (h w) -> p h w", w=HD))
                    nc.gpsimd.memset(dst[:, :, HD:VW], 1.0)
                yield (0.0, evac)

            def kq_unit(kind, p, ib):
                w_all = wk_all if kind == "k" else wq_all
                xt = (xk_t if kind == "k" else xq_t)[ib]
                dst = (kT_sb if kind == "k" else qT_sb)[p]
                ps = projp.tile([BLK, IBW], F32, tag="pp", name=f"{kind}ps{p}_{ib}")
                for d in range(NDT):
                    rhs = xt[:, d * IBW:(d + 1) * IBW]

                    def mm(d=d, rhs=rhs):
                        nc.tensor.matmul(
                            ps,
                            w_all[:, d * FB + p * BLK:d * FB + (p + 1) * BLK],
                            rhs,
                            start=(d == 0),
                            stop=(d == NDT - 1),
                        )
                    yield (MM512, mm)

                def evac():
                    nc.vector.tensor_copy(dst[:, ib * IBW:(ib + 1) * IBW], ps)
                yield (0.0, evac)

            pending_dmas = []

            def flush_dmas():
                while pending_dmas:
                    pending_dmas.pop(0)()

            def out_unit(st, nb):
                # issue the previous unit's store first - its data is ready
                # by now, so the DMA never parks on the SP queue waiting.
                yield (0.0, flush_dmas)
                po = projp.tile([BLK, IBW], F32, tag="pp", name=f"po{st}_{nb}")
                for p in range(NPAIR):
                    def mm(p=p):
                        nc.tensor.matmul(
                            po,
                            ctxT_sb[p][:, st * BLK:(st + 1) * BLK],
                            wo_all[:, p * D + nb * IBW:p * D + (nb + 1) * IBW],
                            start=(p == 0),
                            stop=(p == NPAIR - 1),
                        )
                    yield (MM512, mm)

                def store():
                    ot = outp.tile([BLK, IBW], F32, tag="ot", name=f"ot{st}_{nb}")
                    nc.vector.tensor_copy(ot, po)
                    pending_dmas.append(lambda ot=ot: nc.sync.dma_start(
                        out=out[st * BLK:(st + 1) * BLK, nb * IBW:(nb + 1) * IBW],
                        in_=ot,
                    ))
                yield (0.0, store)

            IB_ORDER = [0, 1, 3, 2]
            feeder = Feeder()
            v_added = set()
            k_added = set()
            for ib in IB_ORDER:
                for p in range(NPAIR):
                    # att(ib) reads kT columns for every j-tile <= 4*ib+3,
                    # i.e. k-projections of ALL column-blocks <= ib.
                    for jb in range(ib + 1):
                        if (p, jb) not in k_added:
                            k_added.add((p, jb))
                            feeder.add(f"k{p}_{jb}", kq_unit("k", p, jb))
                    feeder.add(f"q{p}_{ib}", kq_unit("q", p, ib))
                for jt in range(4 * ib + 4):
                    if jt not in v_added:
                        v_added.add(jt)
                        feeder.add(f"v{jt}", v_unit(jt))

            # ---- attention slots (i-block major) -------------------------
            pending_tp = []

            def flush_tp():
                while pending_tp:
                    pending_tp.pop(0)()

            # fill the PE queue with projection units before the first
            # exp-gated slot so early DMA latency is hidden.
            feeder.spend(9000.0, cap=9000.0)
            for ib in IB_ORDER:
                jts = plan[ib]
                nj = len(jts)
                ces = chunk_es[ib]
                for p in range(NPAIR):
                    flush_tp()
                    for jb in range(ib + 1):
                        feeder.drain_until(f"k{p}_{jb}")
                    feeder.drain_until(f"q{p}_{ib}")

                    ctA = ctxp.tile([BLK, 2 * 2 * VW], F32, tag="ctx",
                                    name=f"cA{p}_{ib}")
                    ctB = ctxp.tile([BLK, 2 * 2 * VW], F32, tag="ctx",
                                    name=f"cB{p}_{ib}")
                    ctx_t = (ctA, ctA, ctB, ctB)
                    # one PSUM accumulation group per tile (= per 2KB zero
                    # region): start on the first matmul emitted into it,
                    # stop on the last.
                    group_total = [0, 0]
                    for ic in range(NCH):
                        group_total[ic // 2] += 2 * len(ces[ic])
                    group_count = [0, 0]

                    sc_t = {}

                    def emit_scores(e, p=p, ib=ib, jts=jts, sc_t=sc_t):
                        jt, c0, c1, mixed = jts[e]
                        sc = scp.tile([BLK, 2 * IBW], F32, tag="sc",
                                      name=f"s{p}_{ib}_{jt}")
                        nc.tensor.matmul(
                            sc[:, c0:c1],
                            kT_sb[p][0:HD, jt * BLK:(jt + 1) * BLK],
                            qT_sb[p][0:HD, ib * IBW + c0:ib * IBW + c1],
                            start=True,
                            stop=True,
                        )
                        nc.tensor.matmul(
                            sc[:, IBW:IBW + (c1 - c0)],
                            kT_sb[p][HD:BLK, jt * BLK:(jt + 1) * BLK],
                            qT_sb[p][HD:BLK, ib * IBW + c0:ib * IBW + c1],
                            start=True,
                            stop=True,
                            tile_position=(HD, 0),
                        )
                        sc_t[e] = sc

                    def emit_tail(e, p=p, ib=ib, jts=jts, sc_t=sc_t,
                                  ctx_t=ctx_t, ces=ces):
                        jt, c0, c1, mixed = jts[e]
                        w = c1 - c0
                        sc = sc_t.pop(e)
                        at = atp.tile([BLK, 2 * IBW], BF, tag="at",
                                      name=f"a{p}_{ib}_{jt}")
                        nc.scalar.activation(
                            out=at[:, c0:IBW + w],
                            in_=sc[:, c0:IBW + w],
                            func=mybir.ActivationFunctionType.Exp,
                            scale=float(EXP_SCALE),
                        )
                        for k in mixed:
                            slot = mixed_slots[(jt, 4 * ib + k)]
                            vs = valid_sb[:, slot * BLK:(slot + 1) * BLK]
                            nc.vector.tensor_mul(
                                at[:, k * BLK:(k + 1) * BLK],
                                at[:, k * BLK:(k + 1) * BLK],
                                vs,
                            )
                            h1c = IBW + k * BLK - c0
                            nc.vector.tensor_mul(
                                at[:, h1c:h1c + BLK], at[:, h1c:h1c + BLK], vs
                            )
                        vv = v_sb[jt].rearrange("p (h w) -> p h w", w=VW)
                        for ic in range(c0 // BLK, c1 // BLK):
                            ct = ctx_t[ic]
                            gi = ic // 2
                            g0 = 2 * (ic % 2)
                            for h in range(2):
                                if h == 0:
                                    lh = at[:, ic * BLK:(ic + 1) * BLK]
                                else:
                                    base = IBW + ic * BLK - c0
                                    lh = at[:, base:base + BLK]
                                nc.tensor.matmul(
                                    ct[:, (g0 + h) * VW:(g0 + h + 1) * VW],
                                    lh,
                                    vv[:, 2 * p + h, :],
                                    start=(group_count[gi] == 0),
                                    stop=(group_count[gi] == group_total[gi] - 1),
                                )
                                group_count[gi] += 1

                    emit_scores(0)
                    for e in range(nj):
                        jt, c0, c1, mixed = jts[e]
                        w = c1 - c0
                        if e + 1 < nj:
                            emit_scores(e + 1)
                            nc0, nc1 = jts[e + 1][1], jts[e + 1][2]
                            pe_next = 2 * (nc1 - nc0) * PE_NS_PER_COL
                        else:
                            pe_next = 0.0
                        nch_e = (c1 - c0) // BLK
                        act_e = (IBW + w - c0) * ACT_NS_PER_COL + ACT_FIXED
                        pe_e = pe_next + nch_e * 2 * VW * PE_NS_PER_COL
                        bonus = 1500.0 if e == 0 else 0.0
                        feeder.spend(act_e - pe_e + bonus)
                        feeder.drain_until(f"v{jt}")
                        emit_tail(e)

                    # evacuate raw ctx PSUM to SBUF immediately (frees the
                    # bank for the next slot's group), then normalize +
                    # xbar-transpose into pair-stacked ctxT off the critical
                    # path.
                    for tix, ct in ((0, ctA), (1, ctB)):
                        ctf = rawp.tile([BLK, 2 * 2 * VW], F32, tag="ctf",
                                        name=f"ctf{p}_{ib}_{tix}")
                        nc.vector.tensor_copy(ctf, ct)
                        rcp = rcpp.tile([BLK, 4], F32, tag="rcp",
                                        name=f"rc{p}_{ib}_{tix}")
                        dens = ctf.rearrange("p (c x) -> p c x", x=VW)[:, :, HD]
                        nc.vector.reciprocal(out=rcp, in_=dens)
                        for icl in range(2):
                            ic = 2 * tix + icl
                            cn = normp.tile([BLK, 2 * HD], BF, tag="cn",
                                            name=f"cn{p}_{ib}_{ic}")
                            for h in range(2):
                                g = 2 * icl + h
                                nc.vector.tensor_scalar_mul(
                                    cn[:, h * HD:(h + 1) * HD],
                                    ctf[:, g * VW:g * VW + HD],
                                    rcp[:, g:g + 1],
                                )
                            pending_tp.append(
                                lambda p=p, ib=ib, ic=ic, cn=cn:
                                nc.sync.dma_start_transpose(
                                    out=ctxT_sb[p][:, ib * IBW + ic * BLK:
                                                   ib * IBW + (ic + 1) * BLK],
                                    in_=cn,
                                ))

                # output projection for this i-block becomes filler work
                for st in range(4 * ib, 4 * ib + 4):
                    for nb in range(2):
                        feeder.add(f"o{st}_{nb}", out_unit(st, nb))

            flush_tp()
            feeder.drain_all()
            flush_dmas()

    nc.compile()
    return nc


_BUILD_CACHE: dict = {}


def _get_nc(mask: np.ndarray):
    states = classify_mask(mask)
    plan, mixed_slots = build_plan(states)
    key = plan_key(plan, mixed_slots)
    if key not in _BUILD_CACHE:
        _BUILD_CACHE[key] = (build_nc(plan, mixed_slots), plan, mixed_slots)
    return _BUILD_CACHE[key]


def _make_in_maps(xq, xk, xv, mask, W_q, W_k, W_v, W_o, mixed_slots):
    nvb = max(1, len(mixed_slots))
    vt = np.zeros((nvb, BLK, BLK), BF16)
    m = np.asarray(mask)
    for (jt, it), slot in mixed_slots.items():
        vt[slot] = (~m[it * BLK:(it + 1) * BLK, jt * BLK:(jt + 1) * BLK]).T.astype(BF16)
    xT = {}
    for b in range(B):
        xqTb = np.asarray(xq[b]).T.astype(BF16)
        xkTb = np.asarray(xk[b]).T.astype(BF16)
        xvTb = np.asarray(xv[b]).T.astype(BF16)
        # slab layout: [jt, (d-chunk, partition), j-col]
        xvSb = np.ascontiguousarray(
            xvTb.reshape(NDT, BLK, NJT, BLK).transpose(2, 0, 1, 3)
        ).reshape(NJT, NDT * BLK, BLK)
        xT[b] = (xqTb, xkTb, xvSb)
    in_maps = []
    for c in range(8):
        b, g = c // G, c % G
        cols = slice(g * FB, (g + 1) * FB)
        in_maps.append(
            {
                "xqT": xT[b][0],
                "xkT": xT[b][1],
                "xvS": xT[b][2],
                "wq": np.asarray(W_q)[:, cols].astype(BF16),
                "wk": np.asarray(W_k)[:, cols].astype(BF16),
                "wv": np.asarray(W_v)[:, cols].astype(BF16),
                "wo": np.asarray(W_o)[cols, :].astype(BF16),
                "validT": vt,
            }
        )
    return in_maps


PROFILE = False
last_hw_exec_ns = None


def kernel(xq, xk, xv, mask, W_q, W_k, W_v, W_o):
    global last_hw_exec_ns
    from concourse import bass_utils

    nc, plan, mixed_slots = _get_nc(mask)
    in_maps = _make_in_maps(xq, xk, xv, mask, W_q, W_k, W_v, W_o, mixed_slots)
    res = bass_utils.run_bass_kernel_spmd(
        nc, in_maps, core_ids=list(range(8))
    )
    if res.exec_time_ns:
        last_hw_exec_ns = res.exec_time_ns
    out = np.empty((B, S, D), np.float32)
    for b in range(B):
        out[b] = res.results[2 * b]["out"] + res.results[2 * b + 1]["out"]
    return out
